# revision 18
# baseline (speedup 1.0000x reference)
"""Trainium2 Bass kernel for unmasked scaled-dot-product attention.

Problem: q, k, v all [4096, 512] fp32.
  out = softmax(q @ k.T / sqrt(512)) @ v

Strategy (8 NeuronCores, SPMD):
  - Shard q by rows: core c takes rows [c*512, (c+1)*512). k, v replicated.
  - Host pre-transposes (free numpy work) so every device matmul gets
    natural layouts:
      qT_c = (q_c / sqrt(512)).T            [512(d), 512(s)]
      kT   = k.T                            [512(d), 4096(t)]
      v                                     [4096(t), 512(e)]
  - Device, per t-tile (128 keys) of 32:
      scoresT[t,s] = kT_tile.T @ qT   (4 accumulating matmuls over d-chunks)
      expT = exp(scoresT)             (ScalarE; no max subtraction --
                                       scores are ~N(0,1) after scaling, so
                                       exp is comfortably in fp16 range)
      outT[e,s] += v_tile.T @ expT    (4 matmuls, accumulated in PSUM)
      den_acc[t,s] += expT            (DVE fp32 accumulate; ~270ns/tile,
                                       interleaves with the tail copies --
                                       a ones-matmul would cost 512 PE
                                       cycles per tile = 6.9us total)
  - Host: den[s] = den_acc.sum(axis=0); out_c = (outT_c / den).T

All matmuls in fp16: 1 cycle/row on the PE, 216 ns/MM at N=512 --
the streaming roofline (256 MMs = 55.3us, ~80% of exec time). fp8
(DoubleRow) was evaluated and rejected: e4m3 quantization of either
exp-weights or v gives ~4-5% max rel error (matmul upcasts fp8 to
e6m3 -- 3 mantissa bits regardless of format -- and the output is a
diffuse weighted average, so per-element ~4% noise lands directly on
the max-rel-err metric), far above the 2e-2 gate. fp16 measures ~6e-4.

Input DMA (the head): all transfers on this runtime land in ONE
hardware pipe (~330 GB/s/core) and in-flight transfers share it
round-robin per-packet, NOT FIFO -- descriptors fan out across the 16
DMA engines concurrently. Queuing the whole input up front therefore
dilutes the urgent head bytes ~8-way (measured: first full q-tile at
~14us). Fix: only qT (512KB) + kT tile0 (128KB) are issued eagerly;
everything else is a dependency CHAIN (each dma_start carries an
explicit dep on a predecessor via add_dep_helper, so its trigger parks
on the sync queue until the predecessor's completion semaphore fires).
At most ~3 streams are ever in flight, in strict need order, sized to
the PE's 1.7us/tile consumption cadence (completion semaphores are
per-transfer, so early transfers are small, later ones big).

Head: the first input bytes cannot reach SBUF before ~8.4us (fixed
~6.2us engine-queue preamble + ~0.7us descriptor write + ~1.5us DGE
kickoff), so ~33 dummy matmuls on memset data warm the HAM clock gate
(PE at 1.2GHz until ~3.4us of sustained activity) while the DMA runs;
the real stream then opens at ~10.5us already at 2.4GHz.

Tail: PSUM evacuated as fp16, split DVE/ACT per bank so copies
pipeline behind each bank's stop matmul; outputs ship as THREE grouped
DMAs (banks 0-1, banks 2-3, den16) pre-parked on the two hardware DGE
rings (sync + scalar) -- trigger instructions wait on the copy
semaphores, so each fires the instant its data lands. gpsimd runs
nothing but the two startup memsets (its software DGE completes ~3us
late and its exit drain gates the final barrier).
"""

import math
import os

import numpy as np

S = 4096      # sequence length (queries == keys)
D = 512       # head dim
N_CORES = 8
SH = S // N_CORES          # query rows per core (512)
P = 128                    # partitions
DC = D // P                # d-chunks (4)
TT = S // P                # t-tiles (32)
ET = D // P                # e-tiles of the output dim (4)

_cache = {}


def _build():
    import concourse.bacc as bacc
    import concourse.tile as tile
    import concourse.mybir as mybir
    from concourse.tile_rust import add_dep_helper

    f32 = mybir.dt.float32
    f16 = mybir.dt.float16

    nc = bacc.Bacc("TRN2", target_bir_lowering=False, debug=False,
                   num_devices=N_CORES)

    qT_d = nc.dram_tensor("qT", [D, SH], f16, kind="ExternalInput")
    # kT is pre-interleaved on the host to [p, t-block, c, u]: every DMA
    # line is then >=1KB contiguous (the natural [D, S] layout gives 256B
    # lines for a t-block slice, which measured ~40% lower DMA rate).
    kT_d = nc.dram_tensor("kT", [P, TT * DC * P], f16, kind="ExternalInput")
    v_d = nc.dram_tensor("v", [S, D], f16, kind="ExternalInput")
    outT_d = nc.dram_tensor("outT", [D, SH], f16, kind="ExternalOutput")
    # fp16 denominator partials (values ~50-4000, 5e-4 rel err -- far
    # inside the tolerance), cast on the DVE before DMA-out.
    dacc_d = nc.dram_tensor("dacc", [P, SH], f16, kind="ExternalOutput")

    # Partition-major views: iteration order matches the SBUF tile layout
    # so one dma_start can move many chunks at once.
    kT_r = kT_d.ap().rearrange("p (t c u) -> p t c u", c=DC, u=P)  # [128,32,4,128]
    qT_r = qT_d.ap().rearrange("(c p) s -> p c s", p=P)       # [128,4,512]
    v_r = v_d.ap().rearrange("(t p) e -> p t e", p=P)         # [128,32,512]
    outT_r = outT_d.ap().rearrange("(e p) s -> p e s", p=P)   # [128,4,512]

    with tile.TileContext(nc) as tc:
        with (
            tc.tile_pool(name="big", bufs=1) as big,
            tc.tile_pool(name="ep", bufs=6) as ep,
            tc.tile_pool(name="outs", bufs=1) as outs,
            tc.tile_pool(name="ps", bufs=3, space="PSUM") as ps,
            tc.tile_pool(name="po", bufs=1, space="PSUM") as po,
        ):
            kT_sb = big.tile([P, TT, DC, P], f16, tag="kT")
            qT_sb = big.tile([P, DC, SH], f16, tag="qT")
            v_sb = big.tile([P, TT, D], f16, tag="v")
            den_acc = big.tile([P, SH], f32, tag="dacc")

            # --- input DMA: eager head, then a bounded-concurrency chain.
            # The head (qT in 4 chunks so QK(0)'s matmuls start on the
            # first-landed chunk, plus kT tile0) goes out ungated.
            # Everything else is ONE interleaved need-order chain with
            # dependency DISTANCE 3: transfer N's trigger parks on the
            # sync queue until transfer N-3 completes. That keeps 3-4
            # streams in flight at all times -- enough concurrency for
            # the pipe's full ~330GB/s (a strict 1-deep chain measured
            # ~120GB/s/stream plus ~1.2us of dead kickoff per link,
            # starving the PE mid-stream), but bounded so the head
            # bytes aren't diluted 8-way like an up-front blast.
            # Two independent ladders on two rings, so a stalled v-link
            # can never head-of-line-block a k-link (the queues execute
            # triggers in order; in the single-queue variant one v
            # trigger with an unsatisfied wait measured a 2.3us PE
            # stall). k rides the sync hardware ring; v rides gpsimd's
            # software DGE -- its ~1-3us extra completion latency is
            # fine because AV trails QK by 2 tiles (+4us of slack).
            d_qc = [nc.sync.dma_start(qT_sb[:, c, :], qT_r[:, c, :])
                    for c in range(DC)]
            d_k0 = nc.scalar.dma_start(kT_sb[:, 0, :, :], kT_r[:, 0, :, :])

            def ladder(eng, items, deps, reason):
                emitted = []
                for n, (dst, src) in enumerate(items):
                    ins = eng.dma_start(dst, src)
                    dep = deps[n](emitted)
                    add_dep_helper(ins.ins, dep.ins, sync=True, reason=reason)
                    emitted.append(ins)
                return emitted

            ks = ladder(nc.sync, [
                (kT_sb[:, 1:3, :, :],   kT_r[:, 1:3, :, :]),
                (kT_sb[:, 3:5, :, :],   kT_r[:, 3:5, :, :]),
                (kT_sb[:, 5:8, :, :],   kT_r[:, 5:8, :, :]),
                (kT_sb[:, 8:12, :, :],  kT_r[:, 8:12, :, :]),
                (kT_sb[:, 12:17, :, :], kT_r[:, 12:17, :, :]),
                (kT_sb[:, 17:23, :, :], kT_r[:, 17:23, :, :]),
                (kT_sb[:, 23:32, :, :], kT_r[:, 23:32, :, :]),
            ], [
                lambda e: d_k0,
                lambda e: e[0],
                lambda e: e[1],
                lambda e: e[0],
                lambda e: e[1],
                lambda e: e[2],
                lambda e: e[3],
            ], "k ladder")
            # Every v-link is seeded off a HARDWARE-ring completion (qc
            # or k ladder): the software DGE's completion notification
            # runs ~1-3us late, so chaining v->v would compound that
            # latency per hop (measured as 2us PE stalls at tiles 2-6).
            ladder(nc.gpsimd, [
                (v_sb[:, 0:2, :],    v_r[:, 0:2, :]),
                (v_sb[:, 2:4, :],    v_r[:, 2:4, :]),
                (v_sb[:, 4:7, :],    v_r[:, 4:7, :]),
                (v_sb[:, 7:11, :],   v_r[:, 7:11, :]),
                (v_sb[:, 11:16, :],  v_r[:, 11:16, :]),
                (v_sb[:, 16:22, :],  v_r[:, 16:22, :]),
                (v_sb[:, 22:32, :],  v_r[:, 22:32, :]),
            ], [
                lambda e: d_qc[3],
                lambda e: ks[0],
                lambda e: ks[1],
                lambda e: ks[2],
                lambda e: ks[3],
                lambda e: ks[4],
                lambda e: ks[5],
            ], "v ladder")

            out_ps = [po.tile([P, SH], f32, tag=f"o{e}", name=f"o{e}")
                      for e in range(ET)]
            # Dedicated PSUM bank for the warmup/bridge dummies so their
            # accumulation group can stay open into tile 2 without
            # touching the real output banks.
            warm_ps = po.tile([P, P], f32, tag="warm_ps")

            # PE warmup while the head DMA is in flight. ~33 small N=128
            # dummy matmuls on memset data keep the PE busy from ~7us so
            # the HAM clock-gate (needs ~3.4us of sustained activity)
            # lifts the PE to 2.4GHz right as the real data arrives
            # (~10.7us). wz's memset goes FIRST on the gpsimd queue,
            # whose user code starts earliest (~6.2us). (The exec-time
            # clock anchors even earlier regardless: Bass.__init__'s
            # const-AP memsets run on gpsimd before any user code.)
            wz = big.tile([P, P], f16, tag="warm")
            nc.gpsimd.memset(wz[:], 0.0)
            nc.gpsimd.memset(den_acc[:], 0.0)
            warm_n = [0]

            def emit_warm(n):
                for _ in range(n):
                    nc.tensor.matmul(
                        warm_ps[:],
                        wz[:],
                        wz[:],
                        start=(warm_n[0] == 0),
                        stop=False,
                    )
                    warm_n[0] += 1

            emit_warm(36)

            # Software pipeline with lag 2: emit QK(ti)+exp(ti) two
            # iterations ahead of AV(ti), so the ScalarE exp of tile ti
            # has ~2 QK-groups of slack before the PE needs it.
            LAG = 2
            ex_q = {}
            exp_ins = {}

            def emit_qk(ti, bridge=0, close_warm=False):
                # bridge: dummy matmuls woven between this tile's QK
                # matmuls as insurance against DMA arrival jitter (a
                # short PE idle is harmless for the HAM -- it needs a
                # full 3.4us idle window to re-throttle -- so only tile
                # 0 carries any).
                sc = ps.tile([P, SH], f32, tag="sc", name=f"sc{ti}")
                for c in range(DC):
                    nc.tensor.matmul(
                        sc[:],
                        kT_sb[:, ti, c, :],
                        qT_sb[:, c, :],
                        start=(c == 0),
                        stop=(c == DC - 1),
                    )
                    emit_warm(bridge)
                if close_warm:
                    nc.tensor.matmul(
                        warm_ps[:], wz[:], wz[:],
                        start=False, stop=True,
                    )
                ex = ep.tile([P, SH], f16, tag="ex", name=f"ex{ti}")
                exp_ins[ti] = nc.scalar.activation(
                    ex[:], sc[:], mybir.ActivationFunctionType.Exp,
                )
                # Denominator partials on the DVE (~270ns/tile; DVE is
                # otherwise idle until the tail) -- keeps gpsimd empty
                # so its slow exit drain never gates the final barrier.
                nc.vector.tensor_add(den_acc[:], den_acc[:], ex[:])
                ex_q[ti] = ex

            def emit_av(ti):
                ex = ex_q.pop(ti)
                for e in range(ET):
                    nc.tensor.matmul(
                        out_ps[e][:],
                        v_sb[:, ti, e * P:(e + 1) * P],
                        ex[:],
                        start=(ti == 0),
                        stop=False,
                    )

            # Main loop covers AV(0..27); the last 4 tiles' AV matmuls
            # are regrouped PER BANK below so bank e's accumulation
            # stops 4*(3-e) matmuls before the stream ends -- its
            # PSUM->SBUF copy and output DMA then pipeline inside the
            # final matmuls instead of all four banks stopping on the
            # very last instruction (measured ~2us off the tail).
            REG = 4
            for ti in range(TT):
                if ti <= 3:
                    emit_qk(ti, bridge=1)
                elif ti == 4:
                    emit_qk(ti, close_warm=True)
                else:
                    emit_qk(ti)
                if ti >= LAG and ti - LAG < TT - REG:
                    emit_av(ti - LAG)
            for e in range(ET):
                for ti in range(TT - REG, TT):
                    nc.tensor.matmul(
                        out_ps[e][:],
                        v_sb[:, ti, e * P:(e + 1) * P],
                        ex_q[ti][:],
                        start=False,
                        stop=(ti == TT - 1),
                    )

            # Tail: per-bank PSUM->SBUF fp16 copies split across DVE
            # and ACT so bank e's copy pipelines right behind its stop
            # matmul; den16 cast slotted between banks 1 and 2 on the
            # DVE (its input is ready ~12 MMs before the stream ends).
            # Outputs ship as three grouped DMAs whose triggers are
            # pre-parked on the two hardware rings (sync: den16 then
            # banks 0-1; scalar: banks 2-3 after its ACT half-copies).
            # A 128-byte ring-warmer DMA fires on each ring ~3 tiles
            # before the end (dep-chained to exp(29)): the first DMA on
            # a ring that's been idle pays ~1.2us of DGE kickoff, and
            # the warmer absorbs it off the critical path.
            outT_sb = outs.tile([P, ET, SH], f16, tag="outT")
            den16 = outs.tile([P, SH], f16, tag="den16")
            warm_d = nc.dram_tensor("warm_d", [1, 64], f16, kind="Internal")
            w1 = nc.sync.dma_start(warm_d.ap()[:], wz[0:1, 0:64])
            add_dep_helper(w1.ins, exp_ins[TT - 3].ins, sync=True,
                           reason="sync ring warmer")
            H2 = 224   # DVE half is smaller: it also carries the den cast
            nc.vector.tensor_copy(den16[:], den_acc[:])
            for e in range(ET):
                nc.vector.tensor_copy(
                    outT_sb[:, e, 0:H2], out_ps[e][:, 0:H2])
                nc.scalar.activation(
                    outT_sb[:, e, H2:SH], out_ps[e][:, H2:SH],
                    mybir.ActivationFunctionType.Copy,
                )
            # All four output DMAs ride the sync ring back-to-back: it
            # has been streaming since the warmer/dacc, so none of them
            # pays the ~1.2us cold-DGE kickoff a fresh ring would.
            nc.sync.dma_start(dacc_d.ap()[:], den16[:])
            nc.sync.dma_start(outT_r[:, 0:2, :], outT_sb[:, 0:2, :])
            nc.sync.dma_start(outT_r[:, 2, :], outT_sb[:, 2, :])
            nc.sync.dma_start(outT_r[:, 3, :], outT_sb[:, 3, :])

    nc.compile()
    return nc


def _get_nc():
    if "nc" not in _cache:
        _cache["nc"] = _build()
    return _cache["nc"]


def kernel(q: np.ndarray, k: np.ndarray, v: np.ndarray) -> np.ndarray:
    from concourse import bass_utils

    assert q.shape == (S, D) and k.shape == (S, D) and v.shape == (S, D)
    scale = 1.0 / math.sqrt(D)

    qs = (np.asarray(q, dtype=np.float32) * scale).astype(np.float16)
    kT = np.asarray(k, dtype=np.float32).T.astype(np.float16)   # [D, S]
    # Interleave kT to [p, t-block, c, u] (see _build) and flatten to
    # [128, 32*4*128] so every DMA line is >=1KB contiguous.
    kTi = np.ascontiguousarray(
        kT.reshape(DC, P, TT, P).transpose(1, 2, 0, 3).reshape(P, TT * DC * P)
    )
    vc = np.ascontiguousarray(np.asarray(v, dtype=np.float32).astype(np.float16))

    in_maps = []
    for c in range(N_CORES):
        qT_c = np.ascontiguousarray(qs[c * SH:(c + 1) * SH].T)
        in_maps.append({"qT": qT_c, "kT": kTi, "v": vc})

    nc = _get_nc()
    trace = bool(int(os.environ.get("KERNEL_TRACE", "0")))
    res = bass_utils.run_bass_kernel_spmd(
        nc, in_maps, core_ids=list(range(N_CORES)), trace=trace,
    )
    if trace:
        print(f"HW exec time: {res.exec_time_ns} ns")
        _cache["last_result"] = res

    out = np.empty((S, D), dtype=np.float32)
    for c in range(N_CORES):
        outT = res.results[c]["outT"].astype(np.float32)   # [512(e), 512(s)]
        den = res.results[c]["dacc"].astype(np.float64).sum(axis=0)  # [512(s)]
        out[c * SH:(c + 1) * SH] = (outT / den[None, :].astype(np.float32)).T
    return out


# revision 20
# speedup vs baseline: 1.0228x; 1.0228x over previous
"""Trainium2 Bass kernel for unmasked scaled-dot-product attention.

Problem: q, k, v all [4096, 512] fp32.
  out = softmax(q @ k.T / sqrt(512)) @ v

Strategy (8 NeuronCores, SPMD):
  - Shard q by rows: core c takes rows [c*512, (c+1)*512). k, v replicated.
  - Host pre-transposes (free numpy work) so every device matmul gets
    natural layouts:
      qT_c = (q_c / sqrt(512)).T            [512(d), 512(s)]
      kT   = k.T                            [512(d), 4096(t)]
      v                                     [4096(t), 512(e)]
  - Device, per t-tile (128 keys) of 32:
      scoresT[t,s] = kT_tile.T @ qT   (4 accumulating matmuls over d-chunks)
      expT = exp(scoresT)             (ScalarE; no max subtraction --
                                       scores are ~N(0,1) after scaling, so
                                       exp is comfortably in fp16 range)
      outT[e,s] += v_tile.T @ expT    (4 matmuls, accumulated in PSUM)
      den_acc[t,s] += expT            (DVE fp32 accumulate; ~270ns/tile,
                                       interleaves with the tail copies --
                                       a ones-matmul would cost 512 PE
                                       cycles per tile = 6.9us total)
  - Host: den[s] = den_acc.sum(axis=0); out_c = (outT_c / den).T

All matmuls in fp16: 1 cycle/row on the PE, 216 ns/MM at N=512 --
the streaming roofline (256 MMs = 55.3us, ~80% of exec time). fp8
(DoubleRow) was evaluated and rejected: e4m3 quantization of either
exp-weights or v gives ~4-5% max rel error (matmul upcasts fp8 to
e6m3 -- 3 mantissa bits regardless of format -- and the output is a
diffuse weighted average, so per-element ~4% noise lands directly on
the max-rel-err metric), far above the 2e-2 gate. fp16 measures ~6e-4.

Input DMA (the head): all transfers on this runtime land in ONE
hardware pipe (~330 GB/s/core) and in-flight transfers share it
round-robin per-packet, NOT FIFO -- descriptors fan out across the 16
DMA engines concurrently. Queuing the whole input up front therefore
dilutes the urgent head bytes ~8-way (measured: first full q-tile at
~14us). Fix: only qT (512KB) + kT tile0 (128KB) are issued eagerly;
everything else is a dependency CHAIN (each dma_start carries an
explicit dep on a predecessor via add_dep_helper, so its trigger parks
on the sync queue until the predecessor's completion semaphore fires).
At most ~3 streams are ever in flight, in strict need order, sized to
the PE's 1.7us/tile consumption cadence (completion semaphores are
per-transfer, so early transfers are small, later ones big).

Head: the first input bytes cannot reach SBUF before ~8.4us (fixed
~6.2us engine-queue preamble + ~0.7us descriptor write + ~1.5us DGE
kickoff), so ~33 dummy matmuls on memset data warm the HAM clock gate
(PE at 1.2GHz until ~3.4us of sustained activity) while the DMA runs;
the real stream then opens at ~10.5us already at 2.4GHz.

Tail: PSUM evacuated as fp16, split DVE/ACT per bank so copies
pipeline behind each bank's stop matmul; outputs ship as THREE grouped
DMAs (banks 0-1, banks 2-3, den16) pre-parked on the two hardware DGE
rings (sync + scalar) -- trigger instructions wait on the copy
semaphores, so each fires the instant its data lands. gpsimd runs
nothing but the two startup memsets (its software DGE completes ~3us
late and its exit drain gates the final barrier).
"""

import math
import os

import numpy as np

S = 4096      # sequence length (queries == keys)
D = 512       # head dim
N_CORES = 8
SH = S // N_CORES          # query rows per core (512)
P = 128                    # partitions
DC = D // P                # d-chunks (4)
TT = S // P                # t-tiles (32)
ET = D // P                # e-tiles of the output dim (4)

_cache = {}


def _build():
    import concourse.bacc as bacc
    import concourse.tile as tile
    import concourse.mybir as mybir
    from concourse.tile_rust import add_dep_helper

    f32 = mybir.dt.float32
    f16 = mybir.dt.float16

    nc = bacc.Bacc("TRN2", target_bir_lowering=False, debug=False,
                   num_devices=N_CORES)

    qT_d = nc.dram_tensor("qT", [D, SH], f16, kind="ExternalInput")
    # kT is pre-interleaved on the host to [p, t-block, c, u]: every DMA
    # line is then >=1KB contiguous (the natural [D, S] layout gives 256B
    # lines for a t-block slice, which measured ~40% lower DMA rate).
    kT_d = nc.dram_tensor("kT", [P, TT * DC * P], f16, kind="ExternalInput")
    v_d = nc.dram_tensor("v", [S, D], f16, kind="ExternalInput")
    outT_d = nc.dram_tensor("outT", [D, SH], f16, kind="ExternalOutput")
    # fp16 denominator partials (values ~50-4000, 5e-4 rel err -- far
    # inside the tolerance), cast on the DVE before DMA-out.
    dacc_d = nc.dram_tensor("dacc", [P, SH], f16, kind="ExternalOutput")

    # Partition-major views: iteration order matches the SBUF tile layout
    # so one dma_start can move many chunks at once.
    kT_r = kT_d.ap().rearrange("p (t c u) -> p t c u", c=DC, u=P)  # [128,32,4,128]
    qT_r = qT_d.ap().rearrange("(c p) s -> p c s", p=P)       # [128,4,512]
    v_r = v_d.ap().rearrange("(t p) e -> p t e", p=P)         # [128,32,512]
    outT_r = outT_d.ap().rearrange("(e p) s -> p e s", p=P)   # [128,4,512]

    with tile.TileContext(nc) as tc:
        with (
            tc.tile_pool(name="big", bufs=1) as big,
            tc.tile_pool(name="ep", bufs=6) as ep,
            tc.tile_pool(name="outs", bufs=1) as outs,
            tc.tile_pool(name="ps", bufs=3, space="PSUM") as ps,
            tc.tile_pool(name="po", bufs=1, space="PSUM") as po,
        ):
            kT_sb = big.tile([P, TT, DC, P], f16, tag="kT")
            qT_sb = big.tile([P, DC, SH], f16, tag="qT")
            v_sb = big.tile([P, TT, D], f16, tag="v")
            den_acc = big.tile([P, SH], f32, tag="dacc")

            # --- input DMA: eager head, then a bounded-concurrency chain.
            # The head (qT in 4 chunks so QK(0)'s matmuls start on the
            # first-landed chunk, plus kT tile0) goes out ungated.
            # Everything else is ONE interleaved need-order chain with
            # dependency DISTANCE 3: transfer N's trigger parks on the
            # sync queue until transfer N-3 completes. That keeps 3-4
            # streams in flight at all times -- enough concurrency for
            # the pipe's full ~330GB/s (a strict 1-deep chain measured
            # ~120GB/s/stream plus ~1.2us of dead kickoff per link,
            # starving the PE mid-stream), but bounded so the head
            # bytes aren't diluted 8-way like an up-front blast.
            # Two independent ladders on two rings, so a stalled v-link
            # can never head-of-line-block a k-link (the queues execute
            # triggers in order; in the single-queue variant one v
            # trigger with an unsatisfied wait measured a 2.3us PE
            # stall). k rides the sync hardware ring; v rides gpsimd's
            # software DGE -- its ~1-3us extra completion latency is
            # fine because AV trails QK by 2 tiles (+4us of slack).
            d_qc = [nc.sync.dma_start(qT_sb[:, c, :], qT_r[:, c, :])
                    for c in range(DC)]
            d_k0 = nc.scalar.dma_start(kT_sb[:, 0, :, :], kT_r[:, 0, :, :])

            def ladder(eng, items, deps, reason):
                emitted = []
                for n, (dst, src) in enumerate(items):
                    ins = eng.dma_start(dst, src)
                    dep = deps[n](emitted)
                    add_dep_helper(ins.ins, dep.ins, sync=True, reason=reason)
                    emitted.append(ins)
                return emitted

            ks = ladder(nc.sync, [
                (kT_sb[:, 1:3, :, :],   kT_r[:, 1:3, :, :]),
                (kT_sb[:, 3:5, :, :],   kT_r[:, 3:5, :, :]),
                (kT_sb[:, 5:8, :, :],   kT_r[:, 5:8, :, :]),
                (kT_sb[:, 8:12, :, :],  kT_r[:, 8:12, :, :]),
                (kT_sb[:, 12:17, :, :], kT_r[:, 12:17, :, :]),
                (kT_sb[:, 17:23, :, :], kT_r[:, 17:23, :, :]),
                (kT_sb[:, 23:32, :, :], kT_r[:, 23:32, :, :]),
            ], [
                lambda e: d_k0,
                lambda e: e[0],
                lambda e: e[1],
                lambda e: e[0],
                lambda e: e[1],
                lambda e: e[2],
                lambda e: e[3],
            ], "k ladder")
            # v rides the SCALAR hardware ring (gpsimd's software DGE
            # measured ~55GB/s per transfer -- a 512KB group took 10us
            # and stalled the PE 4us). The scalar queue also runs the
            # exps, so each v trigger is emitted INSIDE the tile loop
            # (see below) at a point where its wait -- seeded off a
            # k-ladder completion -- is already satisfied when the
            # queue reaches it; a parked trigger would head-of-line
            # block every exp behind it.
            v_plan = {
                0: (v_sb[:, 0:2, :],   v_r[:, 0:2, :],   d_qc[3]),
                1: (v_sb[:, 2:4, :],   v_r[:, 2:4, :],   ks[0]),
                2: (v_sb[:, 4:7, :],   v_r[:, 4:7, :],   ks[1]),
                4: (v_sb[:, 7:11, :],  v_r[:, 7:11, :],  ks[2]),
                6: (v_sb[:, 11:16, :], v_r[:, 11:16, :], ks[3]),
                8: (v_sb[:, 16:22, :], v_r[:, 16:22, :], ks[4]),
                10: (v_sb[:, 22:32, :], v_r[:, 22:32, :], ks[5]),
            }

            out_ps = [po.tile([P, SH], f32, tag=f"o{e}", name=f"o{e}")
                      for e in range(ET)]
            # Dedicated PSUM bank for the warmup/bridge dummies so their
            # accumulation group can stay open into tile 2 without
            # touching the real output banks.
            warm_ps = po.tile([P, P], f32, tag="warm_ps")

            # PE warmup while the head DMA is in flight. ~33 small N=128
            # dummy matmuls on memset data keep the PE busy from ~7us so
            # the HAM clock-gate (needs ~3.4us of sustained activity)
            # lifts the PE to 2.4GHz right as the real data arrives
            # (~10.7us). wz's memset goes FIRST on the gpsimd queue,
            # whose user code starts earliest (~6.2us). (The exec-time
            # clock anchors even earlier regardless: Bass.__init__'s
            # const-AP memsets run on gpsimd before any user code.)
            wz = big.tile([P, P], f16, tag="warm")
            nc.gpsimd.memset(wz[:], 0.0)
            nc.gpsimd.memset(den_acc[:], 0.0)
            warm_n = [0]

            def emit_warm(n):
                for _ in range(n):
                    nc.tensor.matmul(
                        warm_ps[:],
                        wz[:],
                        wz[:],
                        start=(warm_n[0] == 0),
                        stop=False,
                    )
                    warm_n[0] += 1

            emit_warm(36)

            # Software pipeline with lag 2: emit QK(ti)+exp(ti) two
            # iterations ahead of AV(ti), so the ScalarE exp of tile ti
            # has ~2 QK-groups of slack before the PE needs it.
            LAG = 2
            ex_q = {}
            exp_ins = {}

            def emit_qk(ti, bridge=0, close_warm=False):
                # bridge: dummy matmuls woven between this tile's QK
                # matmuls as insurance against DMA arrival jitter (a
                # short PE idle is harmless for the HAM -- it needs a
                # full 3.4us idle window to re-throttle -- so only tile
                # 0 carries any).
                sc = ps.tile([P, SH], f32, tag="sc", name=f"sc{ti}")
                for c in range(DC):
                    nc.tensor.matmul(
                        sc[:],
                        kT_sb[:, ti, c, :],
                        qT_sb[:, c, :],
                        start=(c == 0),
                        stop=(c == DC - 1),
                    )
                    emit_warm(bridge)
                if close_warm:
                    nc.tensor.matmul(
                        warm_ps[:], wz[:], wz[:],
                        start=False, stop=True,
                    )
                ex = ep.tile([P, SH], f16, tag="ex", name=f"ex{ti}")
                exp_ins[ti] = nc.scalar.activation(
                    ex[:], sc[:], mybir.ActivationFunctionType.Exp,
                )
                # Denominator partials on the DVE (~270ns/tile; DVE is
                # otherwise idle until the tail) -- keeps gpsimd empty
                # so its slow exit drain never gates the final barrier.
                nc.vector.tensor_add(den_acc[:], den_acc[:], ex[:])
                ex_q[ti] = ex

            def emit_av(ti):
                ex = ex_q.pop(ti)
                for e in range(ET):
                    nc.tensor.matmul(
                        out_ps[e][:],
                        v_sb[:, ti, e * P:(e + 1) * P],
                        ex[:],
                        start=(ti == 0),
                        stop=False,
                    )

            # Main loop covers AV(0..27); the last 4 tiles' AV matmuls
            # are regrouped PER BANK below so bank e's accumulation
            # stops 4*(3-e) matmuls before the stream ends -- its
            # PSUM->SBUF copy and output DMA then pipeline inside the
            # final matmuls instead of all four banks stopping on the
            # very last instruction (measured ~2us off the tail).
            REG = 4
            for ti in range(TT):
                if ti <= 3:
                    emit_qk(ti, bridge=1)
                elif ti == 4:
                    emit_qk(ti, close_warm=True)
                else:
                    emit_qk(ti)
                if ti in v_plan:
                    dst, src, dep = v_plan[ti]
                    ins = nc.scalar.dma_start(dst, src)
                    add_dep_helper(ins.ins, dep.ins, sync=True,
                                   reason="v trigger (interleaved)")
                if ti >= LAG and ti - LAG < TT - REG:
                    emit_av(ti - LAG)
            for e in range(ET):
                for ti in range(TT - REG, TT):
                    nc.tensor.matmul(
                        out_ps[e][:],
                        v_sb[:, ti, e * P:(e + 1) * P],
                        ex_q[ti][:],
                        start=False,
                        stop=(ti == TT - 1),
                    )

            # Tail: per-bank PSUM->SBUF fp16 copies split across DVE
            # and ACT so bank e's copy pipelines right behind its stop
            # matmul; den16 cast slotted between banks 1 and 2 on the
            # DVE (its input is ready ~12 MMs before the stream ends).
            # Outputs ship as three grouped DMAs whose triggers are
            # pre-parked on the two hardware rings (sync: den16 then
            # banks 0-1; scalar: banks 2-3 after its ACT half-copies).
            # A 128-byte ring-warmer DMA fires on each ring ~3 tiles
            # before the end (dep-chained to exp(29)): the first DMA on
            # a ring that's been idle pays ~1.2us of DGE kickoff, and
            # the warmer absorbs it off the critical path.
            outT_sb = outs.tile([P, ET, SH], f16, tag="outT")
            den16 = outs.tile([P, SH], f16, tag="den16")
            warm_d = nc.dram_tensor("warm_d", [1, 64], f16, kind="Internal")
            w1 = nc.sync.dma_start(warm_d.ap()[:], wz[0:1, 0:64])
            add_dep_helper(w1.ins, exp_ins[TT - 3].ins, sync=True,
                           reason="sync ring warmer")
            H2 = 224   # DVE half is smaller: it also carries the den cast
            nc.vector.tensor_copy(den16[:], den_acc[:])
            for e in range(ET):
                nc.vector.tensor_copy(
                    outT_sb[:, e, 0:H2], out_ps[e][:, 0:H2])
                nc.scalar.activation(
                    outT_sb[:, e, H2:SH], out_ps[e][:, H2:SH],
                    mybir.ActivationFunctionType.Copy,
                )
            # All four output DMAs ride the sync ring back-to-back: it
            # has been streaming since the warmer/dacc, so none of them
            # pays the ~1.2us cold-DGE kickoff a fresh ring would.
            nc.sync.dma_start(dacc_d.ap()[:], den16[:])
            nc.sync.dma_start(outT_r[:, 0:2, :], outT_sb[:, 0:2, :])
            nc.sync.dma_start(outT_r[:, 2, :], outT_sb[:, 2, :])
            nc.sync.dma_start(outT_r[:, 3, :], outT_sb[:, 3, :])

    nc.compile()
    return nc


def _get_nc():
    if "nc" not in _cache:
        _cache["nc"] = _build()
    return _cache["nc"]


def kernel(q: np.ndarray, k: np.ndarray, v: np.ndarray) -> np.ndarray:
    from concourse import bass_utils

    assert q.shape == (S, D) and k.shape == (S, D) and v.shape == (S, D)
    scale = 1.0 / math.sqrt(D)

    qs = (np.asarray(q, dtype=np.float32) * scale).astype(np.float16)
    kT = np.asarray(k, dtype=np.float32).T.astype(np.float16)   # [D, S]
    # Interleave kT to [p, t-block, c, u] (see _build) and flatten to
    # [128, 32*4*128] so every DMA line is >=1KB contiguous.
    kTi = np.ascontiguousarray(
        kT.reshape(DC, P, TT, P).transpose(1, 2, 0, 3).reshape(P, TT * DC * P)
    )
    vc = np.ascontiguousarray(np.asarray(v, dtype=np.float32).astype(np.float16))

    in_maps = []
    for c in range(N_CORES):
        qT_c = np.ascontiguousarray(qs[c * SH:(c + 1) * SH].T)
        in_maps.append({"qT": qT_c, "kT": kTi, "v": vc})

    nc = _get_nc()
    trace = bool(int(os.environ.get("KERNEL_TRACE", "0")))
    res = bass_utils.run_bass_kernel_spmd(
        nc, in_maps, core_ids=list(range(N_CORES)), trace=trace,
    )
    if trace:
        print(f"HW exec time: {res.exec_time_ns} ns")
        _cache["last_result"] = res

    out = np.empty((S, D), dtype=np.float32)
    for c in range(N_CORES):
        outT = res.results[c]["outT"].astype(np.float32)   # [512(e), 512(s)]
        den = res.results[c]["dacc"].astype(np.float64).sum(axis=0)  # [512(s)]
        out[c * SH:(c + 1) * SH] = (outT / den[None, :].astype(np.float32)).T
    return out
